# revision 20
# baseline (speedup 1.0000x reference)
"""Bahdanau-style attention forward on 8 TRN2 NeuronCores.

Per-batch data parallel: batch b -> core b. Each core computes
  S = Q @ K^T          (2048x2048)
  A = softmax(S)       (row softmax with max subtraction)
  C = A @ V            (bf16)
and returns both A (attn) and C (context).

Score matmul precision scheme (~fp32-quality at ~1.5 bf16 passes):
  Q = Qh + Ql, K = Kh + Kl  (bf16 hi + bf16 lo)
  S*512 = (512*Qh)@Kh   [bf16 main pass]
        + (4*Qh)@(128*Kl) + (128*Ql)@(4*Kh)   [one fp8 DoubleRow pass]
  all accumulated in one PSUM group; exp() folds the 1/512 via its scale arg.

Self-contained: hardcodes shapes B=8, Lq=Lk=2048, D=1024, f32 I/O.
"""

import sys

sys.path.insert(0, "/opt/trn_rl_repo")

import numpy as np

B = 8
LQ = 2048
LK = 2048
D = 1024
P = 128
NQ = LQ // P  # 16 q tiles
NK = LK // P  # 16 kv tiles
ND = D // P  # 8 contraction chunks
NKG = 4  # kv column groups for mm1 rhs (each 512 wide)
KG = LK // NKG  # 512
SC = 512.0  # main-pass score scale (exact power of two)

# set by test harness to enable neuron-profile trace
TRACE = False
# timing-attribution multipliers (leave at 1 for correctness)
MAIN_MULT = 1
CORR_MULT = 1
SKIP_ATTN_STORE = False
INTERLEAVE_MM1 = False
DEDUPE_LDW = True
LAST_RESULTS = None

_CACHE = {}


def _dedupe_ldweights(nc, mybir):
    """Remove InstLdweights that reload the exact weights already in the PE
    array (same AP/dtype/mode, no waits/updates). The PE keeps the stationary
    operand across matmuls, so consecutive same-weight matmuls need one load.
    Runs after TileContext exit (final per-engine order) and before finalize.
    """
    removed = 0
    for blk in nc.main_func.blocks:
        insts = list(blk.instructions)
        last_sig = None
        keep = []
        changed = False
        for inst in insts:
            t = type(inst).__name__
            if t == "InstLdweights":
                a = inst.ins[0]
                sig = (
                    a.memref,
                    a.offset,
                    tuple(map(tuple, a.ap)),
                    str(a.dtype),
                    str(inst.perf_mode),
                    bool(inst.is_transpose),
                )
                if sig == last_sig and not inst.has_wait() and not inst.has_update():
                    removed += 1
                    changed = True
                    continue
                last_sig = sig
            elif t == "InstMatmult":
                pass  # consumes, does not change, the loaded weights
            elif inst.is_executable() and inst.engine == mybir.EngineType.PE:
                last_sig = None
            keep.append(inst)
        if changed:
            blk.instructions = keep
    return removed


def _build_nc(reps=1):
    import concourse.bass as bass
    import concourse.tile as tile
    from concourse import bacc, mybir
    from contextlib import nullcontext

    FP = mybir.dt.float32
    BF = mybir.dt.bfloat16
    F8 = mybir.dt.float8e4
    AX = mybir.AxisListType
    AF = mybir.ActivationFunctionType
    DR = mybir.MatmulPerfMode.DoubleRow

    nc = bacc.Bacc(None, target_bir_lowering=False, debug=False)
    q_in = nc.declare_dram_parameter("q_in", [LQ, D], FP, isOutput=False)
    kv_in = nc.declare_dram_parameter("kv_in", [LK, D], FP, isOutput=False)
    attn_out = nc.declare_dram_parameter("attn", [LQ, LK], FP, isOutput=True)
    ctx_out = nc.declare_dram_parameter("context", [LQ, D], FP, isOutput=True)

    with tile.TileContext(nc) as tc:
        from contextlib import ExitStack

        with ExitStack() as ctx:
            kvload = ctx.enter_context(tc.tile_pool(name="kvload", bufs=3))
            klscr = ctx.enter_context(tc.tile_pool(name="klscr", bufs=2))
            kltp = ctx.enter_context(tc.tile_pool(name="kltp", bufs=2))
            qload = ctx.enter_context(tc.tile_pool(name="qload", bufs=2))
            qsplit = ctx.enter_context(tc.tile_pool(name="qsplit", bufs=2))
            qtp = ctx.enter_context(tc.tile_pool(name="qtp", bufs=2))
            res = ctx.enter_context(tc.tile_pool(name="res", bufs=1))
            epool = ctx.enter_context(tc.tile_pool(name="epool", bufs=2))
            abfp = ctx.enter_context(tc.tile_pool(name="abfp", bufs=2))
            atp = ctx.enter_context(tc.tile_pool(name="atp", bufs=2))
            cp = ctx.enter_context(tc.tile_pool(name="cp", bufs=2))
            stats = ctx.enter_context(tc.tile_pool(name="stats", bufs=3))
            spsum = ctx.enter_context(tc.tile_pool(name="spsum", bufs=1, space="PSUM"))
            cpsum = ctx.enter_context(tc.tile_pool(name="cpsum", bufs=2, space="PSUM"))

            # Residents: V bf16 (= Kh), Kh^T bf16 per group, fp8 [Kl;Kh]^T per group
            kh = res.tile([P, NK, D], BF, name="kh", tag="kh")
            khT = [
                res.tile([P, ND, KG], BF, name=f"khT{g}", tag=f"khT{g}")
                for g in range(NKG)
            ]
            k8T = [
                res.tile([P, ND, 2, KG], F8, name=f"k8T{g}", tag=f"k8T{g}")
                for g in range(NKG)
            ]

            def emit_kv_tile(i):
                g, c = i // (NK // NKG), i % (NK // NKG)
                kvt = kvload.tile([P, D], FP, name="kvt", tag="kvt")
                nc.sync.dma_start(kvt[:], kv_in[i * P : (i + 1) * P, :])
                nc.vector.tensor_copy(kh[:, i, :], kvt[:])
                klt = klscr.tile([P, D], BF, name="klt", tag="klt")
                nc.vector.tensor_sub(klt[:], kvt[:], kh[:, i, :])
                nc.sync.dma_start(
                    khT[g][:, :, c * P : (c + 1) * P], kh[:, i, :], transpose=True
                )
                kltT = kltp.tile([P, ND, P], BF, name="kltT", tag="kltT")
                nc.sync.dma_start(kltT[:], klt[:], transpose=True)
                # fp8 slot 0: 128*Kl^T (DVE) ; slot 1: 4*Kh^T (ACT)
                nc.vector.tensor_scalar_mul(
                    k8T[g][:, :, 0, c * P : (c + 1) * P], kltT[:], 128.0
                )
                nc.scalar.activation(
                    k8T[g][:, :, 1, c * P : (c + 1) * P],
                    khT[g][:, :, c * P : (c + 1) * P],
                    AF.Copy,
                    scale=4.0,
                )

            def emit_qchain(i):
                qt = qload.tile([P, D], FP, name="qt", tag="qt")
                nc.sync.dma_start(qt[:], q_in[i * P : (i + 1) * P, :])
                qh = qsplit.tile([P, D], BF, name="qh", tag="qh")
                ql = qsplit.tile([P, D], BF, name="ql", tag="ql")
                qs = qsplit.tile([P, D], BF, name="qs", tag="qs")
                nc.vector.tensor_copy(qh[:], qt[:])
                nc.vector.tensor_sub(ql[:], qt[:], qh[:])
                # 512*Qh for the scaled main pass
                nc.scalar.activation(qs[:], qh[:], AF.Copy, scale=SC)
                qsT = qtp.tile([P, ND, P], BF, name="qsT", tag="qsT")
                qlT = qtp.tile([P, ND, P], BF, name="qlT", tag="qlT")
                nc.sync.dma_start(qsT[:], qs[:], transpose=True)
                nc.sync.dma_start(qlT[:], ql[:], transpose=True)
                # fp8 stationary [4*Qh ; 128*Ql]
                q8 = qtp.tile([P, ND, 2, P], F8, name="q8", tag="q8")
                nc.vector.tensor_scalar_mul(q8[:, :, 0, :], qsT[:], 1.0 / 128.0)
                nc.scalar.activation(q8[:, :, 1, :], qlT[:], AF.Copy, scale=128.0)
                return qsT, q8

            def emit_mm1(qsT, q8):
                s = spsum.tile([P, LK], FP, name="spsum_t", tag="spsum_t")
                # j outer, groups inner: 4 consecutive matmuls share the
                # same stationary operand -> one LDWEIGHTS after dedupe.
                # Only legal when each group is a full PSUM bank (512 f32):
                # interleaved accumulation groups must not share a bank.
                assert KG * 4 % 2048 == 0, "group interleave needs bank-aligned groups"
                for m in range(MAIN_MULT):
                    for j in range(ND):
                        for g in range(NKG):
                            nc.tensor.matmul(
                                s[:, g * KG : (g + 1) * KG],
                                lhsT=qsT[:, j, :],
                                rhs=khT[g][:, j, :],
                                start=(m == 0 and j == 0),
                                stop=False,
                            )
                for m in range(CORR_MULT):
                    for j in range(ND):
                        for g in range(NKG):
                            nc.tensor.matmul(
                                s[:, g * KG : (g + 1) * KG],
                                lhsT=q8[:, j, :, :],
                                rhs=k8T[g][:, j, :, :],
                                start=False,
                                stop=(m == CORR_MULT - 1 and j == ND - 1),
                                perf_mode=DR,
                            )
                return s

            def emit_softmax(i, s):
                negmax = stats.tile([P, 1], FP, name="negmax", tag="negmax")
                nc.vector.reduce_max(negmax[:], s[:], axis=AX.X, negate=True)
                nms = stats.tile([P, 1], FP, name="nms", tag="nms")
                nc.vector.tensor_scalar_mul(nms[:], negmax[:], 1.0 / SC)
                e = epool.tile([P, LK], FP, name="e", tag="e")
                zsum = stats.tile([P, 1], FP, name="zsum", tag="zsum")
                nc.scalar.activation(
                    e[:], s[:], AF.Exp, bias=nms[:], scale=1.0 / SC, accum_out=zsum[:]
                )
                inv = stats.tile([P, 1], FP, name="inv", tag="inv")
                nc.vector.reciprocal(inv[:], zsum[:])
                # normalize in place: A = E * (1/Z)
                nc.scalar.activation(e[:], e[:], AF.Copy, bias=0.0, scale=inv[:])
                if not SKIP_ATTN_STORE:
                    nc.sync.dma_start(attn_out[i * P : (i + 1) * P, :], e[:])
                abf = abfp.tile([P, LK], BF, name="abf", tag="abf")
                nc.vector.tensor_copy(abf[:], e[:])
                at = atp.tile([P, NK, P], BF, name="at", tag="at")
                nc.sync.dma_start(at[:], abf[:], transpose=True)
                return at

            def emit_mm2(at):
                c = cpsum.tile([P, D], FP, name="cpsum_t", tag="cpsum_t")
                ncw = max(1, D // 512)
                w = D // ncw
                for j in range(NK):
                    for n in range(ncw):
                        nc.tensor.matmul(
                            c[:, n * w : (n + 1) * w],
                            lhsT=at[:, j, :],
                            rhs=kh[:, j, n * w : (n + 1) * w],
                            start=(j == 0),
                            stop=(j == NK - 1),
                        )
                return c

            def emit_cout(i, c):
                csb = cp.tile([P, D], FP, name="csb", tag="csb")
                nc.scalar.activation(csb[:], c[:], AF.Copy)
                nc.sync.dma_start(ctx_out[i * P : (i + 1) * P, :], csb[:])

            per_g = NK // NKG

            def emit_body(interleave_prep):
                # ---- prologue: first q tile's mm1 (optionally interleaved
                # with kv prep so PE starts as soon as group 0 is ready) ----
                qT0 = emit_qchain(0)
                if not interleave_prep:
                    s = emit_mm1(*qT0)
                else:
                    s = spsum.tile([P, LK], FP, name="spsum_t", tag="spsum_t")
                    for g in range(NKG):
                        for c in range(per_g):
                            emit_kv_tile(g * per_g + c)
                        for j in range(ND):
                            nc.tensor.matmul(
                                s[:, g * KG : (g + 1) * KG],
                                lhsT=qT0[0][:, j, :],
                                rhs=khT[g][:, j, :],
                                start=(j == 0),
                                stop=False,
                            )
                        for j in range(ND):
                            nc.tensor.matmul(
                                s[:, g * KG : (g + 1) * KG],
                                lhsT=qT0[1][:, j, :, :],
                                rhs=k8T[g][:, j, :, :],
                                start=False,
                                stop=(j == ND - 1),
                                perf_mode=DR,
                            )

                # ---- software-pipelined main loop ----
                cprev = None
                for i in range(NQ):
                    if i + 1 < NQ:
                        qT_next = emit_qchain(i + 1)
                    at = emit_softmax(i, s)
                    if i + 1 < NQ:
                        s = emit_mm1(*qT_next)
                    if cprev is not None:
                        emit_cout(i - 1, cprev)
                    cprev = emit_mm2(at)
                emit_cout(NQ - 1, cprev)

            if reps == 1:
                emit_body(interleave_prep=True)
            else:
                for i in range(NK):
                    emit_kv_tile(i)
                with tc.For_i(0, reps, 1):
                    emit_body(interleave_prep=False)

    if DEDUPE_LDW:
        n_removed = _dedupe_ldweights(nc, mybir)
        assert n_removed > 0, "ldweights dedupe found nothing - loop order regressed?"
    nc.finalize()
    return nc


def kernel(output, inputs):
    global LAST_RESULTS
    from concourse.bass_utils import run_bass_kernel_spmd

    if "nc" not in _CACHE:
        _CACHE["nc"] = _build_nc()
    nc = _CACHE["nc"]

    output = np.ascontiguousarray(np.asarray(output, dtype=np.float32))
    inputs = np.ascontiguousarray(np.asarray(inputs, dtype=np.float32))
    in_maps = [{"q_in": output[b], "kv_in": inputs[b]} for b in range(B)]
    r = run_bass_kernel_spmd(nc, in_maps, core_ids=list(range(B)), trace=TRACE)
    LAST_RESULTS = r
    context = np.stack([r.results[b]["context"] for b in range(B)])
    attn = np.stack([r.results[b]["attn"] for b in range(B)])
    return context, attn


# revision 23
# speedup vs baseline: 1.0985x; 1.0985x over previous
"""Bahdanau-style attention forward on 8 TRN2 NeuronCores.

Per-batch data parallel: batch b -> core b. Each core computes
  S = Q @ K^T          (2048x2048)
  A = softmax(S)       (row softmax with max subtraction)
  C = A @ V            (bf16)
and returns both A (attn) and C (context).

Score matmul precision scheme (~fp32-quality at ~1.5 bf16 passes):
  Q = Qh + Ql, K = Kh + Kl  (bf16 hi + bf16 lo)
  S*512 = (512*Qh)@Kh   [bf16 main pass]
        + (4*Qh)@(128*Kl) + (128*Ql)@(4*Kh)   [one fp8 DoubleRow pass]
  all accumulated in one PSUM group; exp() folds the 1/512 via its scale arg.

Self-contained: hardcodes shapes B=8, Lq=Lk=2048, D=1024, f32 I/O.
"""

import sys

sys.path.insert(0, "/opt/trn_rl_repo")

import numpy as np

B = 8
LQ = 2048
LK = 2048
D = 1024
P = 128
NQ = LQ // P  # 16 q tiles
NK = LK // P  # 16 kv tiles
ND = D // P  # 8 contraction chunks
NKG = 4  # kv column groups for mm1 rhs (each 512 wide)
KG = LK // NKG  # 512
SC = 512.0  # main-pass score scale (exact power of two)

# set by test harness to enable neuron-profile trace
TRACE = False
# timing-attribution multipliers (leave at 1 for correctness)
MAIN_MULT = 1
CORR_MULT = 1
SKIP_ATTN_STORE = False
INTERLEAVE_MM1 = False
DEDUPE_LDW = True
LAST_RESULTS = None

_CACHE = {}


def _dedupe_ldweights(nc, mybir):
    """Remove InstLdweights that reload the exact weights already in the PE
    array (same AP/dtype/mode, no waits/updates). The PE keeps the stationary
    operand across matmuls, so consecutive same-weight matmuls need one load.
    Runs after TileContext exit (final per-engine order) and before finalize.
    """
    removed = 0
    for blk in nc.main_func.blocks:
        insts = list(blk.instructions)
        last_sig = None
        keep = []
        changed = False
        for inst in insts:
            t = type(inst).__name__
            if t == "InstLdweights":
                a = inst.ins[0]
                sig = (
                    a.memref,
                    a.offset,
                    tuple(map(tuple, a.ap)),
                    str(a.dtype),
                    str(inst.perf_mode),
                    bool(inst.is_transpose),
                )
                if sig == last_sig and not inst.has_wait() and not inst.has_update():
                    removed += 1
                    changed = True
                    continue
                last_sig = sig
            elif t == "InstMatmult":
                pass  # consumes, does not change, the loaded weights
            elif inst.is_executable() and inst.engine == mybir.EngineType.PE:
                last_sig = None
            keep.append(inst)
        if changed:
            blk.instructions = keep
    return removed


def _build_nc(reps=1):
    import concourse.bass as bass
    import concourse.tile as tile
    from concourse import bacc, mybir
    from contextlib import nullcontext

    FP = mybir.dt.float32
    BF = mybir.dt.bfloat16
    F8 = mybir.dt.float8e4
    AX = mybir.AxisListType
    AF = mybir.ActivationFunctionType
    DR = mybir.MatmulPerfMode.DoubleRow

    nc = bacc.Bacc(None, target_bir_lowering=False, debug=False)
    q_in = nc.declare_dram_parameter("q_in", [LQ, D], FP, isOutput=False)
    kv_in = nc.declare_dram_parameter("kv_in", [LK, D], FP, isOutput=False)
    attn_out = nc.declare_dram_parameter("attn", [LQ, LK], FP, isOutput=True)
    ctx_out = nc.declare_dram_parameter("context", [LQ, D], FP, isOutput=True)

    with tile.TileContext(nc) as tc:
        from contextlib import ExitStack

        with ExitStack() as ctx:
            kvload = ctx.enter_context(tc.tile_pool(name="kvload", bufs=3))
            klscr = ctx.enter_context(tc.tile_pool(name="klscr", bufs=2))
            kltp = ctx.enter_context(tc.tile_pool(name="kltp", bufs=2))
            qload = ctx.enter_context(tc.tile_pool(name="qload", bufs=2))
            qsplit = ctx.enter_context(tc.tile_pool(name="qsplit", bufs=2))
            qtp = ctx.enter_context(tc.tile_pool(name="qtp", bufs=2))
            res = ctx.enter_context(tc.tile_pool(name="res", bufs=1))
            epool = ctx.enter_context(tc.tile_pool(name="epool", bufs=2))
            abfp = ctx.enter_context(tc.tile_pool(name="abfp", bufs=2))
            atp = ctx.enter_context(tc.tile_pool(name="atp", bufs=2))
            cp = ctx.enter_context(tc.tile_pool(name="cp", bufs=2))
            stats = ctx.enter_context(tc.tile_pool(name="stats", bufs=3))
            spsum = ctx.enter_context(tc.tile_pool(name="spsum", bufs=1, space="PSUM"))
            cpsum = ctx.enter_context(tc.tile_pool(name="cpsum", bufs=2, space="PSUM"))

            # Residents: V bf16 (= Kh), Kh^T bf16 per group, fp8 [Kl;Kh]^T per group
            kh = res.tile([P, NK, D], BF, name="kh", tag="kh")
            khT = [
                res.tile([P, ND, KG], BF, name=f"khT{g}", tag=f"khT{g}")
                for g in range(NKG)
            ]
            k8T = [
                res.tile([P, ND, 2, KG], F8, name=f"k8T{g}", tag=f"k8T{g}")
                for g in range(NKG)
            ]

            def emit_kv_tile(i):
                g, c = i // (NK // NKG), i % (NK // NKG)
                kvt = kvload.tile([P, D], FP, name="kvt", tag="kvt")
                nc.sync.dma_start(kvt[:], kv_in[i * P : (i + 1) * P, :])
                nc.vector.tensor_copy(kh[:, i, :], kvt[:])
                klt = klscr.tile([P, D], BF, name="klt", tag="klt")
                nc.vector.tensor_sub(klt[:], kvt[:], kh[:, i, :])
                nc.sync.dma_start(
                    khT[g][:, :, c * P : (c + 1) * P], kh[:, i, :], transpose=True
                )
                kltT = kltp.tile([P, ND, P], BF, name="kltT", tag="kltT")
                nc.sync.dma_start(kltT[:], klt[:], transpose=True)
                # fp8 slot 0: 128*Kl^T (DVE) ; slot 1: 4*Kh^T (ACT)
                nc.vector.tensor_scalar_mul(
                    k8T[g][:, :, 0, c * P : (c + 1) * P], kltT[:], 128.0
                )
                nc.scalar.activation(
                    k8T[g][:, :, 1, c * P : (c + 1) * P],
                    khT[g][:, :, c * P : (c + 1) * P],
                    AF.Copy,
                    scale=4.0,
                )

            def emit_qchain(i):
                qt = qload.tile([P, D], FP, name="qt", tag="qt")
                nc.sync.dma_start(qt[:], q_in[i * P : (i + 1) * P, :])
                qh = qsplit.tile([P, D], BF, name="qh", tag="qh")
                ql = qsplit.tile([P, D], BF, name="ql", tag="ql")
                qs = qsplit.tile([P, D], BF, name="qs", tag="qs")
                nc.vector.tensor_copy(qh[:], qt[:])
                nc.vector.tensor_sub(ql[:], qt[:], qh[:])
                # 512*Qh for the scaled main pass
                nc.scalar.activation(qs[:], qh[:], AF.Copy, scale=SC)
                qsT = qtp.tile([P, ND, P], BF, name="qsT", tag="qsT")
                qlT = qtp.tile([P, ND, P], BF, name="qlT", tag="qlT")
                nc.sync.dma_start(qsT[:], qs[:], transpose=True)
                nc.sync.dma_start(qlT[:], ql[:], transpose=True)
                # fp8 stationary [4*Qh ; 128*Ql]
                q8 = qtp.tile([P, ND, 2, P], F8, name="q8", tag="q8")
                nc.vector.tensor_scalar_mul(q8[:, :, 0, :], qsT[:], 1.0 / 128.0)
                nc.scalar.activation(q8[:, :, 1, :], qlT[:], AF.Copy, scale=128.0)
                return qsT, q8

            def emit_mm1(qsT, q8):
                s = spsum.tile([P, LK], FP, name="spsum_t", tag="spsum_t")
                # j outer, groups inner: 4 consecutive matmuls share the
                # same stationary operand -> one LDWEIGHTS after dedupe.
                # Only legal when each group is a full PSUM bank (512 f32):
                # a bank must hold at most one open accumulation group, so for
                # small/unaligned shapes fall back to fully sequential groups.
                if KG * 4 % 2048 == 0:
                    for m in range(MAIN_MULT):
                        for j in range(ND):
                            for g in range(NKG):
                                nc.tensor.matmul(
                                    s[:, g * KG : (g + 1) * KG],
                                    lhsT=qsT[:, j, :],
                                    rhs=khT[g][:, j, :],
                                    start=(m == 0 and j == 0),
                                    stop=False,
                                )
                    for m in range(CORR_MULT):
                        for j in range(ND):
                            for g in range(NKG):
                                nc.tensor.matmul(
                                    s[:, g * KG : (g + 1) * KG],
                                    lhsT=q8[:, j, :, :],
                                    rhs=k8T[g][:, j, :, :],
                                    start=False,
                                    stop=(m == CORR_MULT - 1 and j == ND - 1),
                                    perf_mode=DR,
                                )
                else:
                    for g in range(NKG):
                        for m in range(MAIN_MULT):
                            for j in range(ND):
                                nc.tensor.matmul(
                                    s[:, g * KG : (g + 1) * KG],
                                    lhsT=qsT[:, j, :],
                                    rhs=khT[g][:, j, :],
                                    start=(m == 0 and j == 0),
                                    stop=False,
                                )
                        for m in range(CORR_MULT):
                            for j in range(ND):
                                nc.tensor.matmul(
                                    s[:, g * KG : (g + 1) * KG],
                                    lhsT=q8[:, j, :, :],
                                    rhs=k8T[g][:, j, :, :],
                                    start=False,
                                    stop=(m == CORR_MULT - 1 and j == ND - 1),
                                    perf_mode=DR,
                                )
                return s

            def emit_softmax(i, s):
                negmax = stats.tile([P, 1], FP, name="negmax", tag="negmax")
                nc.vector.reduce_max(negmax[:], s[:], axis=AX.X, negate=True)
                nms = stats.tile([P, 1], FP, name="nms", tag="nms")
                nc.vector.tensor_scalar_mul(nms[:], negmax[:], 1.0 / SC)
                e = epool.tile([P, LK], FP, name="e", tag="e")
                zsum = stats.tile([P, 1], FP, name="zsum", tag="zsum")
                nc.scalar.activation(
                    e[:], s[:], AF.Exp, bias=nms[:], scale=1.0 / SC, accum_out=zsum[:]
                )
                inv = stats.tile([P, 1], FP, name="inv", tag="inv")
                nc.vector.reciprocal(inv[:], zsum[:])
                # normalize in place: A = E * (1/Z)
                nc.scalar.activation(e[:], e[:], AF.Copy, bias=0.0, scale=inv[:])
                if not SKIP_ATTN_STORE:
                    nc.sync.dma_start(attn_out[i * P : (i + 1) * P, :], e[:])
                abf = abfp.tile([P, LK], BF, name="abf", tag="abf")
                nc.vector.tensor_copy(abf[:], e[:])
                at = atp.tile([P, NK, P], BF, name="at", tag="at")
                nc.sync.dma_start(at[:], abf[:], transpose=True)
                return at

            def emit_mm2(at):
                c = cpsum.tile([P, D], FP, name="cpsum_t", tag="cpsum_t")
                ncw = max(1, D // 512)
                w = D // ncw
                for j in range(NK):
                    for n in range(ncw):
                        nc.tensor.matmul(
                            c[:, n * w : (n + 1) * w],
                            lhsT=at[:, j, :],
                            rhs=kh[:, j, n * w : (n + 1) * w],
                            start=(j == 0),
                            stop=(j == NK - 1),
                        )
                return c

            def emit_cout(i, c):
                csb = cp.tile([P, D], FP, name="csb", tag="csb")
                nc.scalar.activation(csb[:], c[:], AF.Copy)
                nc.sync.dma_start(ctx_out[i * P : (i + 1) * P, :], csb[:])

            per_g = NK // NKG

            def emit_body(interleave_prep):
                # ---- prologue: first q tile's mm1 (optionally interleaved
                # with kv prep so PE starts as soon as group 0 is ready) ----
                qT0 = emit_qchain(0)
                if not interleave_prep:
                    s = emit_mm1(*qT0)
                else:
                    s = spsum.tile([P, LK], FP, name="spsum_t", tag="spsum_t")
                    for g in range(NKG):
                        for c in range(per_g):
                            emit_kv_tile(g * per_g + c)
                        for j in range(ND):
                            nc.tensor.matmul(
                                s[:, g * KG : (g + 1) * KG],
                                lhsT=qT0[0][:, j, :],
                                rhs=khT[g][:, j, :],
                                start=(j == 0),
                                stop=False,
                            )
                        for j in range(ND):
                            nc.tensor.matmul(
                                s[:, g * KG : (g + 1) * KG],
                                lhsT=qT0[1][:, j, :, :],
                                rhs=k8T[g][:, j, :, :],
                                start=False,
                                stop=(j == ND - 1),
                                perf_mode=DR,
                            )

                # ---- software-pipelined main loop ----
                cprev = None
                for i in range(NQ):
                    if i + 1 < NQ:
                        qT_next = emit_qchain(i + 1)
                    at = emit_softmax(i, s)
                    if i + 1 < NQ:
                        s = emit_mm1(*qT_next)
                    if cprev is not None:
                        emit_cout(i - 1, cprev)
                    cprev = emit_mm2(at)
                emit_cout(NQ - 1, cprev)

            if reps == 1:
                emit_body(interleave_prep=True)
            else:
                for i in range(NK):
                    emit_kv_tile(i)
                with tc.For_i(0, reps, 1):
                    emit_body(interleave_prep=False)

    if DEDUPE_LDW:
        n_removed = _dedupe_ldweights(nc, mybir)
        if KG * 4 % 2048 == 0:
            assert n_removed > 0, "ldweights dedupe found nothing - loop order regressed?"
    nc.finalize()
    return nc


def kernel(output, inputs):
    global LAST_RESULTS
    from concourse.bass_utils import run_bass_kernel_spmd

    if "nc" not in _CACHE:
        _CACHE["nc"] = _build_nc()
    nc = _CACHE["nc"]

    output = np.ascontiguousarray(np.asarray(output, dtype=np.float32))
    inputs = np.ascontiguousarray(np.asarray(inputs, dtype=np.float32))
    in_maps = [{"q_in": output[b], "kv_in": inputs[b]} for b in range(B)]
    r = run_bass_kernel_spmd(nc, in_maps, core_ids=list(range(B)), trace=TRACE)
    LAST_RESULTS = r
    context = np.stack([r.results[b]["context"] for b in range(B)])
    attn = np.stack([r.results[b]["attn"] for b in range(B)])
    return context, attn
